# revision 153
# baseline (speedup 1.0000x reference)
"""Bidirectional 2nd-order IIR filter bank (64 channels) on 8 TRN2 NeuronCores.

Block-parallel scan over time (same math as the f64 reference), tuned for the
v1 CoreSim cost model (DMA engine charge = free-dim bytes x 0.3855ns min
500ns on the dispatching engine; compute charge = free-size x cycle_t +
access bubble; matmul charge = out free-size x PE cycle).

Restructure vs the earlier baseline (37.4us -> 27.7us):
  - Phase A (zero-state Toeplitz) runs in fp8e4m3 DoubleRow matmuls
    (0.5 cycles/row): lhsT [64,(i,v8,m)], rhs [64,(i,n)], K = i*64+k2.
    The particular solution carries <1% of output energy for these high-Q
    filters, so fp8 there adds only ~1e-3 rel_l2 (total 1.4e-3, gate 2e-2).
  - xrhs2 and its 16 min-charge reshape DMAs eliminated: the transposed p~
    chunk IS xt_chunk.T @ wp, computed directly as 24 [64,32] PE matmuls
    into PSUM, then copied to SBUF. The comp1 duplicate (needed because an
    accumulation group may not mix lhsT base partitions, and lhsT/rhs bases
    must match) lives in a SEPARATE TILE per comp half: reader-writer deps
    are column-range based (partition-blind), but writer-writer deps are
    tile-granular, so same-tile duplicates would serialize.
  - s0/s1 second-level-scan matmuls are K=64 comp-split groups; comp0/comp1
    accumulate into separate PSUM tiles (mixed-base groups are illegal on
    hw), summed during the PSUM->SBUF stage (copy + tensor_tensor add; only
    one PSUM input per DVE op is allowed).
  - All tables are split into per-consumer-group TILES (wb, wb2, g, wt8 by
    v-half; xt8 per slice) because a reader waits for ALL writers of a
    tile; DMA queues on SP/Pool/Act are deadline-ordered along the critical
    chain xt -> rp -> s0(wb) -> s1(wb2) -> sv -> C(g). Act's queue opens
    with the hoisted 1283ns activation-table load.
  - Output path: PSUM f32 -> SBUF fp16 copies alternate DVE (658ns) and
    Act (612ns) — the only engines that can read PSUM — and are the
    back-half capacity bound; out DMAs rotate SP/Pool. The last vc's copies
    are split across both engines and DMA'd per-slice to shorten the tail.
  - 4 phase-A matmuls parked in spare PSUM banks fill the sv-wait gap; the
    s0/s1 blocks are emitted comp-MAJOR so the comp0 stream (gated only by
    the earlier DVE rp copy) runs while Act's comp1 rp copy lands. The sv
    tail (ss1 copy/add -> psvb transposes -> inb) runs in b-halves to
    pipeline the copy engines with PE, and the A+C loop is SLICE-major:
    s=0 reads only the earliest-ready sv columns, giving the late sv
    writers (ina1, inb b4-7) a 16-slice runway.
Sharding: 128 (direction x channel) independent filters; cores 0-3 forward
channels 0-63, cores 4-7 backward, 16 filters/core, B=8 local. Output DRAM
layout [vc, n, col] (col = b*192 + block) fp16; final transpose to [b, c, t]
and f32 cast on host.
"""

import sys

import numpy as np

if "/opt/trn_rl_repo" not in sys.path:
    sys.path.insert(0, "/opt/trn_rl_repo")

T = 24000
B = 8
C = 64
L = 128
NBLK = 192
TPAD = NBLK * L  # 24576
NCOL = B * NBLK  # 1536
NVC = 16  # filters per core
NSL = 3  # 512-wide column slices
SLW = 512

ALPHA = 2.0 ** 6
BETA = 2.0 ** -13

_PROGRAM_CACHE = {}


# --------------------------------------------------------------------------
# host-side table construction (float64 -> float16, device SBUF layouts)
# --------------------------------------------------------------------------

def _tables_for_channels(a, b, chans):
    """Per-core weight tables for 16 channels. a,b: [64,3] float64."""
    NV = len(chans)
    wt = np.zeros((NV, 128, 128), np.float64)
    wb = np.zeros((NV, 2, 2, 128, 128), np.float64)  # [v, r2, half, (comp,i), j]
    wb2 = np.zeros((NV, 2, 2, 128, 64), np.float64)  # [v, r2, sc, (comp,i), j']
    wp = np.zeros((128, 2 * NV), np.float64)
    ga = np.zeros((32, NV * 128), np.float64)

    for vi, ch in enumerate(chans):
        a1, a2, b0 = a[ch, 1], a[ch, 2], b[ch, 0]
        r = np.sqrt(a2)
        costh = -a1 / (2.0 * r)
        sinth = np.sqrt(max(0.0, 1.0 - costh * costh))
        th = np.arctan2(sinth, costh)

        # impulse response h[m] = b0 * phi(m), phi: homogeneous w/ phi(0)=1
        h = np.zeros(130)
        h[0] = b0
        h[1] = -a1 * b0
        for m in range(2, 130):
            h[m] = -a1 * h[m - 1] - a2 * h[m - 2]

        # Toeplitz lhsT: wt[k, m] = h[m-k] for m >= k
        k_i = np.arange(128)
        d = k_i[None, :] - k_i[:, None]
        wt[vi] = np.where(d >= 0, h[np.clip(d, 0, 129)], 0.0)

        # modal decomposition: Vinv = [[0,1],[1/(r s), -c/s]]
        Vinv = np.array([[0.0, 1.0], [1.0 / (r * sinth), -costh / sinth]])
        wp[:, 2 * vi:2 * vi + 2] = wt[vi][:, [127, 126]] @ Vinv.T

        # g[c, n] = r^{n+2} (cos((n+2)th), sin((n+2)th))
        ks = np.arange(1, 129)
        rk = r ** (ks + 1.0)
        ga[2 * vi, vi * 128:(vi + 1) * 128] = rk * np.cos((ks + 1) * th)
        ga[2 * vi + 1, vi * 128:(vi + 1) * 128] = rk * np.sin((ks + 1) * th)

        # Mpow[q] = S^(128 q): scaled rotations
        qs = np.arange(0, 193)
        rq = r ** (128.0 * qs)
        ang = 128.0 * qs * th
        Mg = np.zeros((2, 2, 193))
        Mg[0, 0] = rq * np.cos(ang)
        Mg[0, 1] = rq * np.sin(ang)
        Mg[1, 0] = -Mg[0, 1]
        Mg[1, 1] = Mg[0, 0]

        ii = np.arange(64)
        jj = np.arange(128)
        j1 = np.arange(64)
        for r2 in range(2):
            for half in range(2):
                dd = jj[None, :] - (64 * half + ii[:, None])
                msk = dd >= 0
                dc = np.clip(dd, 0, 128)
                wb[vi, r2, half, 0:64, :] = np.where(msk, Mg[r2, 0, dc], 0.0)
                wb[vi, r2, half, 64:128, :] = np.where(msk, Mg[r2, 1, dc], 0.0)
            # chunk-2 states directly from p~ of chunks 0/1
            for sc in range(2):
                dd2 = 128 + j1[None, :] - 64 * sc - ii[:, None]
                wb2[vi, r2, sc, 0:64] = Mg[r2, 0, dd2]
                wb2[vi, r2, sc, 64:128] = Mg[r2, 1, dd2]

    import ml_dtypes

    # phase-A Toeplitz in fp8e4m3 DoubleRow layout: [64, (vhalf, i, v8, m)]
    # where K = i*64 + k2. |wt/BETA| <= ~60 fits e4m3 range comfortably.
    wt_s = (wt / BETA).transpose(1, 0, 2)  # [128 (k), v, m]
    wt8 = (
        wt_s.reshape(2, 64, 2, 8, 128)      # [i, k2, vh, v8, m]
        .transpose(1, 2, 0, 3, 4)           # [k2, vh, i, v8, m]
    )

    return {
        "wt8": np.ascontiguousarray(
            wt8.reshape(64, 2 * NV * 128)
        ).astype(ml_dtypes.float8_e4m3),
        "wb": np.ascontiguousarray(
            wb.transpose(3, 0, 1, 2, 4).reshape(128, NV * 512)
        ).astype(np.float16),
        "wb2": np.ascontiguousarray(
            wb2.transpose(3, 0, 1, 2, 4).reshape(128, NV * 256)
        ).astype(np.float16),
        "wp": (wp * ALPHA).astype(np.float16),
        "gall": (ga / (ALPHA * BETA)).astype(np.float16),
    }


# --------------------------------------------------------------------------
# device program
# --------------------------------------------------------------------------

def build_nc():
    """Build + compile the single-core Tile program (same on all 8 cores)."""
    import concourse.bass as bass
    import concourse.tile as tile
    from concourse import bacc, mybir

    f32 = mybir.dt.float32
    f16 = mybir.dt.float16
    f8 = mybir.dt.float8e4

    nc = bacc.Bacc("TRN2", target_bir_lowering=False, debug=False)

    xr_d = nc.dram_tensor("xrhs", [128, NCOL], f16, kind="ExternalInput")
    x8_d = nc.dram_tensor("xrhs8", [64, 2 * NCOL], f8, kind="ExternalInput")
    wt_d = nc.dram_tensor("wt8", [64, 2 * NVC * 128], f8, kind="ExternalInput")
    wb_d = nc.dram_tensor("wb", [128, NVC * 512], f16, kind="ExternalInput")
    wb2_d = nc.dram_tensor("wb2", [128, NVC * 256], f16, kind="ExternalInput")
    wp_d = nc.dram_tensor("wp", [128, 2 * NVC], f16, kind="ExternalInput")
    ga_d = nc.dram_tensor("gall", [32, NVC * 128], f16, kind="ExternalInput")
    id_d = nc.dram_tensor("ident", [128, 128], f16, kind="ExternalInput")
    out_d = nc.dram_tensor("out", [NVC, 128, NCOL], f16, kind="ExternalOutput")

    with tile.TileContext(nc) as tc:
        with (
            tc.tile_pool(name="const", bufs=1) as const,
            tc.tile_pool(name="work", bufs=1) as work,
            tc.tile_pool(name="yout", bufs=18) as yout_pool,
            tc.tile_pool(name="bpsum", bufs=4, space="PSUM") as bpsum,
            tc.tile_pool(name="opsum", bufs=4, space="PSUM") as opsum,
        ):
            # ---- constants into SBUF, striped so each table lands just
            # before its first consumer. Act's queue opens with the hoisted
            # 1283ns activation-table load (for the scalar-engine copies),
            # so nothing startup-critical rides on Act.
            # Tile-granular dependencies: a reader waits for ALL writers of a
            # tile, so tables are split into per-consumer-group tiles loaded
            # in deadline order. The sv critical chain is xt -> pp ->
            # transposes -> s0(wb) -> s1(wb2) -> sv -> C(g).
            xt = const.tile([128, NCOL], f16)
            wp_t = const.tile([128, 2 * NVC], f16)
            id_t = const.tile([128, 128], f16)
            wt8t = [const.tile([64, 2048], f8, name=f"wt8{h}")
                    for h in range(2)]
            xt8t = [const.tile([64, 1024], f8, name=f"xt8{s}")
                    for s in range(3)]
            wbt = [const.tile([128, 4096], f16, name=f"wb{h}")
                   for h in range(2)]
            wb2t = [const.tile([128, 2048], f16, name=f"wb2{h}")
                    for h in range(2)]
            gt = [const.tile([32, 1024], f16, name=f"g{h}")
                  for h in range(2)]

            # SP queue
            nc.sync.dma_start(xt[:, 0:768], xr_d[:, 0:768])
            nc.sync.dma_start(wp_t[:], wp_d[:])
            nc.sync.dma_start(wbt[0][:, 0:1024], wb_d[:, 0:1024])
            nc.sync.dma_start(wbt[0][:, 1024:2048], wb_d[:, 1024:2048])
            nc.sync.dma_start(wbt[1][:, 0:1024], wb_d[:, 4096:5120])
            nc.sync.dma_start(wb2t[1][:, 0:1024], wb2_d[:, 2048:3072])
            nc.sync.dma_start(id_t[:], id_d[:])
            nc.sync.dma_start(gt[0][:], ga_d[:, 0:1024])
            nc.sync.dma_start(xt8t[0][:], x8_d[:, 0:1024])
            nc.sync.dma_start(xt8t[1][:], x8_d[:, 1024:2048])
            nc.sync.dma_start(xt8t[2][:], x8_d[:, 2048:3072])
            # Pool queue
            nc.gpsimd.dma_start(xt[:, 768:1536], xr_d[:, 768:1536])
            nc.gpsimd.dma_start(wbt[0][:, 2048:3072], wb_d[:, 2048:3072])
            nc.gpsimd.dma_start(wbt[0][:, 3072:4096], wb_d[:, 3072:4096])
            nc.gpsimd.dma_start(wb2t[0][:, 0:1024], wb2_d[:, 0:1024])
            nc.gpsimd.dma_start(wb2t[0][:, 1024:2048], wb2_d[:, 1024:2048])
            nc.gpsimd.dma_start(wb2t[1][:, 1024:2048], wb2_d[:, 3072:4096])
            nc.gpsimd.dma_start(gt[1][:], ga_d[:, 1024:2048])
            nc.gpsimd.dma_start(wt8t[0][:, 0:1024], wt_d[:, 0:1024])
            nc.gpsimd.dma_start(wt8t[0][:, 1024:2048], wt_d[:, 1024:2048])
            nc.gpsimd.dma_start(wt8t[1][:, 0:1024], wt_d[:, 2048:3072])
            nc.gpsimd.dma_start(wt8t[1][:, 1024:2048], wt_d[:, 3072:4096])
            # Act queue: three early DMAs + id behind the hoisted act-table
            # load (Act idles until its first chain copy anyway).
            nc.scalar.dma_start(wbt[1][:, 1024:2048], wb_d[:, 5120:6144])
            nc.scalar.dma_start(wbt[1][:, 2048:3072], wb_d[:, 6144:7168])
            nc.scalar.dma_start(wbt[1][:, 3072:4096], wb_d[:, 7168:8192])

            # ---- persistent work tiles
            # rp duplicates live in SEPARATE TILES per comp half: writer-
            # writer dependencies are tile-granular (even at disjoint
            # columns), so sharing a tile would serialize the two copies
            rpAB0 = work.tile([64, 512], f16)
            rpAB1 = work.tile([128, 512], f16)  # rows 64:128 used
            rpC0 = work.tile([64, 256], f16)
            rpC1 = work.tile([128, 256], f16)  # rows 64:128 used
            ss0_all = work.tile([128, 256], f16)  # cols b*32 + v*2 + r2
            ss1_all = work.tile([64, 256], f16)
            sv_all = work.tile([32, NCOL], f16)  # rows 2v+r2, cols b*192+n

            # ---- phase A matmul emission helper (parked early to fill PE
            # dependency-wait gaps during phase B; C-part lands later)
            yo_tiles = {}
            ps_tiles = {}

            wt8_r = [t[:].rearrange("p (i v m) -> p i v m", i=2, v=8, m=128)
                     for t in wt8t]
            xt8_r = [t[:].rearrange("p (i n) -> p i n", i=2, n=SLW)
                     for t in xt8t]

            def emit_A(v, s, pool=None):
                if v not in yo_tiles:
                    if v == NVC - 1:
                        # tail vc: one SBUF tile PER SLICE so each slice's
                        # out DMA waits only its own two half-copies
                        yo_tiles[v] = [
                            yout_pool.tile([128, SLW], f16, tag="y",
                                           name=f"yo{v}_{q}")
                            for q in range(NSL)
                        ]
                    else:
                        yo_tiles[v] = yout_pool.tile(
                            [128, NCOL], f16, tag="y", name=f"yo{v}"
                        )
                if pool is None:
                    pool, tag = opsum, "o"
                elif pool is bpsum:
                    tag = "bp"
                else:
                    tag = "o"
                ps = pool.tile([128, SLW], f32, tag=tag)
                ps_tiles[(v, s)] = ps
                nc.tensor.matmul(
                    ps[:], wt8_r[v // 8][:, :, v % 8, :], xt8_r[s][:],
                    start=True, stop=False,
                    perf_mode=mybir.MatmulPerfMode.DoubleRow,
                )

            A, V = nc.scalar, nc.vector
            yo_copy_eng = (V, A, V, A, V, A)

            def emit_C(v, s):
                sli = slice(s * SLW, (s + 1) * SLW)
                ps = ps_tiles.pop((v, s))
                nc.tensor.matmul(
                    ps[:], gt[v // 8][:, (v % 8) * 128:(v % 8 + 1) * 128],
                    sv_all[:, sli],
                    start=False, stop=True,
                )
                if v == NVC - 1:
                    # tail: split the copy across DVE+Act so the last out
                    # DMA dispatches as early as possible
                    yo = yo_tiles[v][s]
                    V.tensor_copy(yo[:, 0:256], ps[:, 0:256])
                    A.copy(yo[:, 256:512], ps[:, 256:512])
                else:
                    yo = yo_tiles[v]
                    eng = yo_copy_eng[(v * NSL + s) % 6]
                    if eng is nc.scalar:
                        eng.copy(yo[:, sli], ps[:])
                    else:
                        eng.tensor_copy(yo[:, sli], ps[:])

            # ---- rp chunks computed DIRECTLY: the transposed p~ chunk is
            # xt_chunk.T @ wp, one [64,32] matmul per (c, b) — no pp SBUF
            # staging, no PE transposes. The result is duplicated into both
            # partition halves so comp1 matmuls (lhsT base partition 64)
            # have a matching-base rhs.
            rptpA = bpsum.tile([64, 512], f32, tag="bp", name="rptpA")
            rptpB = bpsum.tile([64, 256], f32, tag="bp", name="rptpB")
            for bb in range(B):
                for c in range(2):
                    nc.tensor.matmul(
                        rptpA[:, (c * 8 + bb) * 32:(c * 8 + bb) * 32 + 32],
                        xt[:, bb * 192 + c * 64: bb * 192 + c * 64 + 64],
                        wp_t[:], start=True, stop=True,
                    )
            nc.scalar.copy(rpAB1[64:128, :], rptpA[:])
            nc.vector.tensor_copy(rpAB0[:], rptpA[:])
            for bb in range(B):
                nc.tensor.matmul(
                    rptpB[:, bb * 32:bb * 32 + 32],
                    xt[:, bb * 192 + 128: bb * 192 + 192],
                    wp_t[:], start=True, stop=True,
                )
            nc.vector.tensor_copy(rpC0[:], rptpB[:])
            nc.scalar.copy(rpC1[64:128, :], rptpB[:])
            rp_r = {
                (0, 0): rpAB0[:].rearrange("p (q vr) -> p q vr", q=16, vr=32),
                (0, 1): rpAB1[:].rearrange("p (q vr) -> p q vr", q=16, vr=32),
                (1, 0): rpC0[:].rearrange("p (q vr) -> p q vr", q=8, vr=32),
                (1, 1): rpC1[:].rearrange("p (q vr) -> p q vr", q=8, vr=32),
            }

            def rp(c, comp, v, r2):
                # [64, 8, 1] AP at base partition comp*64: rows i, cols b
                # (stride 32) for fixed (c, 2v+comp)
                rr = rp_r[(1 if c == 2 else 0, comp)]
                q0 = 0 if c == 2 else c * 8
                return rr[comp * 64:comp * 64 + 64,
                          q0:q0 + 8,
                          2 * v + comp:2 * v + comp + 1]

            # ---- s0/s1: comp-split K=64 matmuls. An accumulation group may
            # not mix lhsT base partitions, so comp0 and comp1 accumulate
            # into separate PSUM tiles, summed during the PSUM->SBUF stage
            # (copy from c0, then tensor_tensor add of c1; only one PSUM
            # input is allowed per DVE op).
            s0c = [bpsum.tile([128, 256], f32, tag="bp", name=f"s0c{h}")
                   for h in range(2)]
            s1c = [bpsum.tile([64, 256], f32, tag="bp", name=f"s1c{h}")
                   for h in range(2)]
            # s0 block first -> its PSUM->SBUF chain overlaps the s1 block.
            # comp-MAJOR emission: the comp0 stream needs only the earlier
            # DVE rp copy, so it runs while Act's comp1 rp copy lands.
            for comp in range(2):
                rows = slice(comp * 64, comp * 64 + 64)
                for v in range(NVC):
                    wbv = wbt[v // 8]
                    for r2 in range(2):
                        cs = slice(v * 16 + r2 * 8, v * 16 + r2 * 8 + 8)
                        base = (2 * (v % 8) + r2) * 256
                        nc.tensor.matmul(
                            s0c[comp][:, cs], wbv[rows, base:base + 128],
                            rp(0, comp, v, r2), start=True, stop=False,
                        )
                        nc.tensor.matmul(
                            s0c[comp][:, cs], wbv[rows, base + 128:base + 256],
                            rp(1, comp, v, r2), start=False, stop=True,
                        )

            # batched (v,r2,b) -> (b,v,r2) permute: copy comp0, add comp1
            alu_add = mybir.AluOpType.add
            dst0 = ss0_all[:].rearrange("p (b v r) -> p b v r", b=8, v=16, r=2)
            nc.scalar.copy(
                dst0, s0c[0][:].rearrange("p (v r b) -> p b v r", v=16, r=2, b=8)
            )
            nc.vector.tensor_tensor(
                dst0, dst0,
                s0c[1][:].rearrange("p (v r b) -> p b v r", v=16, r=2, b=8),
                alu_add,
            )

            for comp in range(2):
                rows = slice(comp * 64, comp * 64 + 64)
                for v in range(NVC):
                    wbv = wbt[v // 8]
                    wb2v = wb2t[v // 8]
                    for r2 in range(2):
                        cs = slice(v * 16 + r2 * 8, v * 16 + r2 * 8 + 8)
                        base = (2 * (v % 8) + r2) * 256
                        b2 = (v % 8) * 256 + r2 * 128
                        nc.tensor.matmul(
                            s1c[comp][:, cs], wb2v[rows, b2:b2 + 64],
                            rp(0, comp, v, r2), start=True, stop=False,
                        )
                        nc.tensor.matmul(
                            s1c[comp][:, cs], wb2v[rows, b2 + 64:b2 + 128],
                            rp(1, comp, v, r2), start=False, stop=False,
                        )
                        nc.tensor.matmul(
                            s1c[comp][:, cs], wbv[rows, base:base + 64],
                            rp(2, comp, v, r2), start=False, stop=True,
                        )

            emit_A(0, 0)
            emit_A(1, 0)
            emit_A(2, 0)
            emit_A(3, 0)

            # ---- per-batch transposes into sv layout. The psva half (fed
            # by the ss0 chain, which overlapped the s1 matmul block) is
            # emitted BEFORE the ss1 chain so Act/DVE aren't head-of-line
            # blocked when s1 finishes; the ss1->psvb->inb tail then runs in
            # b-halves to pipeline copy/add with the transposes.
            psva = [bpsum.tile([32, 512], f16, tag="bp", name=f"psva{h}")
                    for h in range(2)]
            psvb = bpsum.tile([32, 512], f16, tag="bp", name="psvb")
            dst = sv_all[:].rearrange("p (b n) -> p b n", b=8, n=192)
            # zero-state col b*192 (only cols never otherwise written)
            nc.vector.memset(dst[:, :, 0:1], 0.0)
            for bb in range(B):
                nc.tensor.transpose(
                    psva[bb // 4][:, (bb % 4) * 128:(bb % 4 + 1) * 128],
                    ss0_all[:, bb * 32:(bb + 1) * 32], id_t[:],
                )
            ina0 = psva[0][:].rearrange("p (b n) -> p b n", b=4, n=128)
            nc.vector.tensor_copy(dst[:, 0:4, 1:129], ina0)
            ina1 = psva[1][:].rearrange("p (b n) -> p b n", b=4, n=128)
            nc.scalar.copy(dst[:, 4:8, 1:129], ina1)

            dst1 = ss1_all[:].rearrange("p (b v r) -> p b v r", b=8, v=16, r=2)
            s1r = [t[:].rearrange("p (v r b) -> p b v r", v=16, r=2, b=8)
                   for t in s1c]
            inb = psvb[:].rearrange("p (b n) -> p b n", b=8, n=64)[:, :, 0:63]
            for h in range(2):
                bs = slice(4 * h, 4 * h + 4)
                nc.scalar.copy(dst1[:, bs], s1r[0][:, bs])
                nc.vector.tensor_tensor(
                    dst1[:, bs], dst1[:, bs], s1r[1][:, bs], alu_add,
                )
                for bb in range(4 * h, 4 * h + 4):
                    nc.tensor.transpose(
                        psvb[:, bb * 64:(bb + 1) * 64],
                        ss1_all[:, bb * 32:(bb + 1) * 32], id_t[0:64, 0:64],
                    )
                nc.vector.tensor_copy(dst[:, bs, 129:192], inb[:, bs])

            # ---- phases A + C per 512-col slice, SLICE-MAJOR: deps are
            # column-range based, and s=0 reads only the earliest-ready sv
            # region (b0-2), so all 16 filters' s=0 slices run while the sv
            # tail writers (ina1/inb b4-7) land. PSUM copies rotate DVE/Act
            # (the only engines that can read PSUM); out DMAs rotate SP/Pool.
            e_out = (nc.sync, nc.gpsimd) * 8
            e_out2 = (nc.gpsimd, nc.sync) * 8
            oeng = (nc.sync, nc.gpsimd, nc.sync)
            # After sv is built the 4 bpsum banks are dead; alternating the
            # stream's PSUM tiles across BOTH pools gives 8 cycling banks so
            # PE never stalls waiting for a copy to free a bank.
            for s in range(NSL):
                for v in range(NVC):
                    if (v, s) not in ps_tiles:
                        emit_A(v, s,
                               pool=bpsum if (v * NSL + s) % 2 else opsum)
                    emit_C(v, s)
                    if v == NVC - 1:
                        # tail vc: per-slice DMA right behind its copy
                        oeng[s].dma_start(
                            out_d[v, :, s * SLW:(s + 1) * SLW],
                            yo_tiles[v][s][:],
                        )
                    elif s == NSL - 1:
                        yo = yo_tiles[v]
                        e_out[v].dma_start(out_d[v, :, 0:768], yo[:, 0:768])
                        e_out2[v].dma_start(out_d[v, :, 768:1536],
                                            yo[:, 768:1536])

    nc.compile()
    return nc


def _get_program():
    if "nc" not in _PROGRAM_CACHE:
        _PROGRAM_CACHE["nc"] = build_nc()
    return _PROGRAM_CACHE["nc"]


# --------------------------------------------------------------------------
# host driver
# --------------------------------------------------------------------------

def make_in_maps(x, a_coeffs, b_coeffs):
    x = np.asarray(x, np.float32)
    a = np.asarray(a_coeffs, np.float64)
    b = np.asarray(b_coeffs, np.float64)
    xf = x[:, 0, :]

    import ml_dtypes

    def to_rhs(x2d):
        xpad = np.zeros((B, TPAD), np.float32)
        xpad[:, :T] = x2d
        xr = np.ascontiguousarray(
            xpad.reshape(B, NBLK, L).transpose(2, 0, 1).reshape(128, NCOL)
        )
        # DoubleRow rhs: [k2, (s, i, n512)] with K = i*64 + k2
        xr8 = np.ascontiguousarray(
            xr.reshape(2, 64, 3, SLW).transpose(1, 2, 0, 3).reshape(64, 2 * NCOL)
        ).astype(ml_dtypes.float8_e4m3)
        return xr.astype(np.float16), xr8

    Xf, X8f = to_rhs(xf)
    Xb, X8b = to_rhs(xf[:, ::-1])
    ident = np.eye(128, dtype=np.float16)

    in_maps = []
    for core in range(8):
        fwd = core < 4
        chans = list(range((core % 4) * NVC, (core % 4) * NVC + NVC))
        tabs = _tables_for_channels(a, b, chans)
        in_maps.append(
            {
                "xrhs": Xf if fwd else Xb,
                "xrhs8": X8f if fwd else X8b,
                "ident": ident,
                **tabs,
            }
        )
    return in_maps


def assemble_output(core_outs):
    y = np.zeros((B, 2 * C, T), np.float32)
    for core in range(8):
        o = np.asarray(core_outs[core]).astype(np.float32) * BETA  # [16, 128, 1536]
        o = o.reshape(NVC, 128, B, NBLK).transpose(2, 0, 3, 1).reshape(B, NVC, TPAD)
        if core < 4:
            y[:, core * NVC:(core + 1) * NVC, :] = o[:, :, :T]
        else:
            y[:, C + (core - 4) * NVC:C + (core - 3) * NVC, :] = o[:, :, :T][:, :, ::-1]
    return y


def kernel(x, a_coeffs, b_coeffs, _trace=False):
    from concourse.bass_utils import run_bass_kernel_spmd

    nc = _get_program()
    in_maps = make_in_maps(x, a_coeffs, b_coeffs)
    res = run_bass_kernel_spmd(
        nc, in_maps, core_ids=list(range(8)), trace=_trace
    )
    y = assemble_output([r["out"] for r in res.results])
    if _trace:
        kernel.last_results = res
    return y


# revision 160
# speedup vs baseline: 1.0291x; 1.0291x over previous
"""Bidirectional 2nd-order IIR filter bank (64 channels) on 8 TRN2 NeuronCores.

Block-parallel scan over time (same math as the f64 reference), tuned for the
v1 CoreSim cost model (DMA engine charge = free-dim bytes x 0.3855ns min
500ns on the dispatching engine; compute charge = free-size x cycle_t +
access bubble; matmul charge = out free-size x PE cycle).

Restructure vs the earlier baseline (37.4us -> 27.7us):
  - Phase A (zero-state Toeplitz) runs in fp8e4m3 DoubleRow matmuls
    (0.5 cycles/row): lhsT [64,(i,v8,m)], rhs [64,(i,n)], K = i*64+k2.
    The particular solution carries <1% of output energy for these high-Q
    filters, so fp8 there adds only ~1e-3 rel_l2 (total 1.4e-3, gate 2e-2).
  - xrhs2 and its 16 min-charge reshape DMAs eliminated: the transposed p~
    chunk IS xt_chunk.T @ wp, computed directly as 24 [64,32] PE matmuls
    into PSUM, then copied to SBUF. The comp1 duplicate (needed because an
    accumulation group may not mix lhsT base partitions, and lhsT/rhs bases
    must match) lives in a SEPARATE TILE per comp half: reader-writer deps
    are column-range based (partition-blind), but writer-writer deps are
    tile-granular, so same-tile duplicates would serialize.
  - s0/s1 second-level-scan matmuls are K=64 comp-split groups; comp0/comp1
    accumulate into separate PSUM tiles (mixed-base groups are illegal on
    hw), summed during the PSUM->SBUF stage (copy + tensor_tensor add; only
    one PSUM input per DVE op is allowed).
  - All tables are split into per-consumer-group TILES (wb, wb2, g, wt8 by
    v-half; xt8 per slice) because a reader waits for ALL writers of a
    tile; DMA queues on SP/Pool/Act are deadline-ordered along the critical
    chain xt -> rp -> s0(wb) -> s1(wb2) -> sv -> C(g). Act's queue opens
    with the hoisted 1283ns activation-table load.
  - Output path: PSUM f32 -> SBUF fp16 copies alternate DVE (658ns) and
    Act (612ns) — the only engines that can read PSUM — and are the
    back-half capacity bound; out DMAs rotate SP/Pool. The last vc's copies
    are split across both engines and DMA'd per-slice to shorten the tail.
  - 4 phase-A matmuls parked in spare PSUM banks fill the sv-wait gap; the
    s0/s1 blocks are emitted comp-MAJOR so the comp0 stream (gated only by
    the earlier DVE rp copy) runs while Act's comp1 rp copy lands. The sv
    tail (ss1 copy/add -> psvb transposes -> inb) runs in b-halves to
    pipeline the copy engines with PE, and the A+C loop is SLICE-major:
    s=0 reads only the earliest-ready sv columns, giving the late sv
    writers (ina1, inb b4-7) a 16-slice runway.
Sharding: 128 (direction x channel) independent filters; cores 0-3 forward
channels 0-63, cores 4-7 backward, 16 filters/core, B=8 local. Output DRAM
layout [vc, n, col] (col = b*192 + block) fp16; final transpose to [b, c, t]
and f32 cast on host.
"""

import sys

import numpy as np

if "/opt/trn_rl_repo" not in sys.path:
    sys.path.insert(0, "/opt/trn_rl_repo")

T = 24000
B = 8
C = 64
L = 128
NBLK = 192
TPAD = NBLK * L  # 24576
NCOL = B * NBLK  # 1536
NVC = 16  # filters per core
NSL = 3  # 512-wide column slices
SLW = 512

ALPHA = 2.0 ** 6
BETA = 2.0 ** -13

_PROGRAM_CACHE = {}


# --------------------------------------------------------------------------
# host-side table construction (float64 -> float16, device SBUF layouts)
# --------------------------------------------------------------------------

def _tables_for_channels(a, b, chans):
    """Per-core weight tables for 16 channels. a,b: [64,3] float64."""
    NV = len(chans)
    wt = np.zeros((NV, 128, 128), np.float64)
    wb = np.zeros((NV, 2, 2, 128, 128), np.float64)  # [v, r2, half, (comp,i), j]
    wb2 = np.zeros((NV, 2, 2, 128, 64), np.float64)  # [v, r2, sc, (comp,i), j']
    wp = np.zeros((128, 2 * NV), np.float64)
    ga = np.zeros((32, NV * 128), np.float64)

    for vi, ch in enumerate(chans):
        a1, a2, b0 = a[ch, 1], a[ch, 2], b[ch, 0]
        r = np.sqrt(a2)
        costh = -a1 / (2.0 * r)
        sinth = np.sqrt(max(0.0, 1.0 - costh * costh))
        th = np.arctan2(sinth, costh)

        # impulse response h[m] = b0 * phi(m), phi: homogeneous w/ phi(0)=1
        h = np.zeros(130)
        h[0] = b0
        h[1] = -a1 * b0
        for m in range(2, 130):
            h[m] = -a1 * h[m - 1] - a2 * h[m - 2]

        # Toeplitz lhsT: wt[k, m] = h[m-k] for m >= k
        k_i = np.arange(128)
        d = k_i[None, :] - k_i[:, None]
        wt[vi] = np.where(d >= 0, h[np.clip(d, 0, 129)], 0.0)

        # modal decomposition: Vinv = [[0,1],[1/(r s), -c/s]]
        Vinv = np.array([[0.0, 1.0], [1.0 / (r * sinth), -costh / sinth]])
        wp[:, 2 * vi:2 * vi + 2] = wt[vi][:, [127, 126]] @ Vinv.T

        # g[c, n] = r^{n+2} (cos((n+2)th), sin((n+2)th))
        ks = np.arange(1, 129)
        rk = r ** (ks + 1.0)
        ga[2 * vi, vi * 128:(vi + 1) * 128] = rk * np.cos((ks + 1) * th)
        ga[2 * vi + 1, vi * 128:(vi + 1) * 128] = rk * np.sin((ks + 1) * th)

        # Mpow[q] = S^(128 q): scaled rotations
        qs = np.arange(0, 193)
        rq = r ** (128.0 * qs)
        ang = 128.0 * qs * th
        Mg = np.zeros((2, 2, 193))
        Mg[0, 0] = rq * np.cos(ang)
        Mg[0, 1] = rq * np.sin(ang)
        Mg[1, 0] = -Mg[0, 1]
        Mg[1, 1] = Mg[0, 0]

        ii = np.arange(64)
        jj = np.arange(128)
        j1 = np.arange(64)
        for r2 in range(2):
            for half in range(2):
                dd = jj[None, :] - (64 * half + ii[:, None])
                msk = dd >= 0
                dc = np.clip(dd, 0, 128)
                wb[vi, r2, half, 0:64, :] = np.where(msk, Mg[r2, 0, dc], 0.0)
                wb[vi, r2, half, 64:128, :] = np.where(msk, Mg[r2, 1, dc], 0.0)
            # chunk-2 states directly from p~ of chunks 0/1
            for sc in range(2):
                dd2 = 128 + j1[None, :] - 64 * sc - ii[:, None]
                wb2[vi, r2, sc, 0:64] = Mg[r2, 0, dd2]
                wb2[vi, r2, sc, 64:128] = Mg[r2, 1, dd2]

    import ml_dtypes

    # phase-A Toeplitz in fp8e4m3 DoubleRow layout: [64, (vhalf, i, v8, m)]
    # where K = i*64 + k2. |wt/BETA| <= ~60 fits e4m3 range comfortably.
    wt_s = (wt / BETA).transpose(1, 0, 2)  # [128 (k), v, m]
    wt8 = (
        wt_s.reshape(2, 64, 2, 8, 128)      # [i, k2, vh, v8, m]
        .transpose(1, 2, 0, 3, 4)           # [k2, vh, i, v8, m]
    )

    return {
        "wt8": np.ascontiguousarray(
            wt8.reshape(64, 2 * NV * 128)
        ).astype(ml_dtypes.float8_e4m3),
        "wb": np.ascontiguousarray(
            wb.transpose(3, 0, 1, 2, 4).reshape(128, NV * 512)
        ).astype(np.float16),
        "wb2": np.ascontiguousarray(
            wb2.transpose(3, 0, 1, 2, 4).reshape(128, NV * 256)
        ).astype(np.float16),
        "wp": (wp * ALPHA).astype(np.float16),
        "gall": (ga / (ALPHA * BETA)).astype(np.float16),
    }


# --------------------------------------------------------------------------
# device program
# --------------------------------------------------------------------------

def build_nc():
    """Build + compile the single-core Tile program (same on all 8 cores)."""
    import concourse.bass as bass
    import concourse.tile as tile
    from concourse import bacc, mybir

    f32 = mybir.dt.float32
    f16 = mybir.dt.float16
    f8 = mybir.dt.float8e4

    nc = bacc.Bacc("TRN2", target_bir_lowering=False, debug=False)

    xr_d = nc.dram_tensor("xrhs", [128, NCOL], f16, kind="ExternalInput")
    x8_d = nc.dram_tensor("xrhs8", [64, 2 * NCOL], f8, kind="ExternalInput")
    wt_d = nc.dram_tensor("wt8", [64, 2 * NVC * 128], f8, kind="ExternalInput")
    wb_d = nc.dram_tensor("wb", [128, NVC * 512], f16, kind="ExternalInput")
    wb2_d = nc.dram_tensor("wb2", [128, NVC * 256], f16, kind="ExternalInput")
    wp_d = nc.dram_tensor("wp", [128, 2 * NVC], f16, kind="ExternalInput")
    ga_d = nc.dram_tensor("gall", [32, NVC * 128], f16, kind="ExternalInput")
    id_d = nc.dram_tensor("ident", [128, 128], f16, kind="ExternalInput")
    out_d = nc.dram_tensor("out", [NVC, 128, NCOL], f16, kind="ExternalOutput")

    with tile.TileContext(nc) as tc:
        with (
            tc.tile_pool(name="const", bufs=1) as const,
            tc.tile_pool(name="work", bufs=1) as work,
            tc.tile_pool(name="yout", bufs=18) as yout_pool,
            tc.tile_pool(name="bpsum", bufs=4, space="PSUM") as bpsum,
            tc.tile_pool(name="opsum", bufs=4, space="PSUM") as opsum,
        ):
            # ---- constants into SBUF, striped so each table lands just
            # before its first consumer. Act's queue opens with the hoisted
            # 1283ns activation-table load (for the scalar-engine copies),
            # so nothing startup-critical rides on Act.
            # Tile-granular dependencies: a reader waits for ALL writers of a
            # tile, so tables are split into per-consumer-group tiles loaded
            # in deadline order. The sv critical chain is xt -> pp ->
            # transposes -> s0(wb) -> s1(wb2) -> sv -> C(g).
            xt = const.tile([128, NCOL], f16)
            wp_t = const.tile([128, 2 * NVC], f16)
            id_t = const.tile([128, 128], f16)
            wt8t = [const.tile([64, 2048], f8, name=f"wt8{h}")
                    for h in range(2)]
            xt8t = [const.tile([64, 1024], f8, name=f"xt8{s}")
                    for s in range(3)]
            wbt = [const.tile([128, 4096], f16, name=f"wb{h}")
                   for h in range(2)]
            wb2t = [const.tile([128, 2048], f16, name=f"wb2{h}")
                    for h in range(2)]
            gt = [const.tile([32, 1024], f16, name=f"g{h}")
                  for h in range(2)]

            # SP queue
            nc.sync.dma_start(xt[:, 0:768], xr_d[:, 0:768])
            nc.sync.dma_start(wp_t[:], wp_d[:])
            nc.sync.dma_start(wbt[0][:, 0:1024], wb_d[:, 0:1024])
            nc.sync.dma_start(wbt[0][:, 1024:2048], wb_d[:, 1024:2048])
            nc.sync.dma_start(wbt[1][:, 0:1024], wb_d[:, 4096:5120])
            nc.sync.dma_start(wb2t[1][:, 0:1024], wb2_d[:, 2048:3072])
            nc.sync.dma_start(id_t[:], id_d[:])
            nc.sync.dma_start(gt[0][:], ga_d[:, 0:1024])
            nc.sync.dma_start(xt8t[0][:], x8_d[:, 0:1024])
            nc.sync.dma_start(xt8t[1][:], x8_d[:, 1024:2048])
            nc.sync.dma_start(xt8t[2][:], x8_d[:, 2048:3072])
            # Pool queue
            nc.gpsimd.dma_start(xt[:, 768:1536], xr_d[:, 768:1536])
            nc.gpsimd.dma_start(wbt[0][:, 2048:3072], wb_d[:, 2048:3072])
            nc.gpsimd.dma_start(wbt[0][:, 3072:4096], wb_d[:, 3072:4096])
            nc.gpsimd.dma_start(wb2t[0][:, 0:1024], wb2_d[:, 0:1024])
            nc.gpsimd.dma_start(wb2t[0][:, 1024:2048], wb2_d[:, 1024:2048])
            nc.gpsimd.dma_start(wb2t[1][:, 1024:2048], wb2_d[:, 3072:4096])
            nc.gpsimd.dma_start(gt[1][:], ga_d[:, 1024:2048])
            nc.gpsimd.dma_start(wt8t[0][:, 0:1024], wt_d[:, 0:1024])
            nc.gpsimd.dma_start(wt8t[0][:, 1024:2048], wt_d[:, 1024:2048])
            nc.gpsimd.dma_start(wt8t[1][:, 0:1024], wt_d[:, 2048:3072])
            nc.gpsimd.dma_start(wt8t[1][:, 1024:2048], wt_d[:, 3072:4096])
            # Act queue: three early DMAs + id behind the hoisted act-table
            # load (Act idles until its first chain copy anyway).
            nc.scalar.dma_start(wbt[1][:, 1024:2048], wb_d[:, 5120:6144])
            nc.scalar.dma_start(wbt[1][:, 2048:3072], wb_d[:, 6144:7168])
            nc.scalar.dma_start(wbt[1][:, 3072:4096], wb_d[:, 7168:8192])

            # ---- persistent work tiles
            # rp duplicates live in SEPARATE TILES per comp half: writer-
            # writer dependencies are tile-granular (even at disjoint
            # columns), so sharing a tile would serialize the two copies
            rpAB0 = work.tile([64, 512], f16)
            rpAB1 = work.tile([128, 512], f16)  # rows 64:128 used
            rpC0 = work.tile([64, 256], f16)
            rpC1 = work.tile([128, 256], f16)  # rows 64:128 used
            ss0_all = work.tile([128, 256], f16)  # cols b*32 + v*2 + r2
            ss1_all = work.tile([64, 256], f16)
            sv_all = work.tile([32, NCOL], f16)  # rows 2v+r2, cols b*192+n

            # ---- phase A matmul emission helper (parked early to fill PE
            # dependency-wait gaps during phase B; C-part lands later)
            yo_tiles = {}
            ps_tiles = {}

            wt8_r = [t[:].rearrange("p (i v m) -> p i v m", i=2, v=8, m=128)
                     for t in wt8t]
            xt8_r = [t[:].rearrange("p (i n) -> p i n", i=2, n=SLW)
                     for t in xt8t]

            def emit_A(v, s, pool=None):
                if v not in yo_tiles:
                    if v == NVC - 1:
                        # tail vc: one SBUF tile PER SLICE so each slice's
                        # out DMA waits only its own two half-copies
                        yo_tiles[v] = [
                            yout_pool.tile([128, SLW], f16, tag="y",
                                           name=f"yo{v}_{q}")
                            for q in range(NSL)
                        ]
                    else:
                        yo_tiles[v] = yout_pool.tile(
                            [128, NCOL], f16, tag="y", name=f"yo{v}"
                        )
                if pool is None:
                    pool, tag = opsum, "o"
                elif pool is bpsum:
                    tag = "bp"
                else:
                    tag = "o"
                ps = pool.tile([128, SLW], f32, tag=tag)
                ps_tiles[(v, s)] = ps
                nc.tensor.matmul(
                    ps[:], wt8_r[v // 8][:, :, v % 8, :], xt8_r[s][:],
                    start=True, stop=False,
                    perf_mode=mybir.MatmulPerfMode.DoubleRow,
                )

            A, V = nc.scalar, nc.vector
            yo_copy_eng = (V, A, V, A, V, A)

            def emit_C(v, s):
                sli = slice(s * SLW, (s + 1) * SLW)
                ps = ps_tiles.pop((v, s))
                nc.tensor.matmul(
                    ps[:], gt[v // 8][:, (v % 8) * 128:(v % 8 + 1) * 128],
                    sv_all[:, sli],
                    start=False, stop=True,
                )
                if v == NVC - 1:
                    # tail: split the copy across DVE+Act so the last out
                    # DMA dispatches as early as possible; s=0 rides DVE
                    # alone to shave Act's (longer) end-of-stream drain
                    yo = yo_tiles[v][s]
                    if s <= 1:
                        A.copy(yo[:, 0:256], ps[:, 0:256])
                    else:
                        V.tensor_copy(yo[:, 0:256], ps[:, 0:256])
                    A.copy(yo[:, 256:512], ps[:, 256:512])
                else:
                    yo = yo_tiles[v]
                    eng = yo_copy_eng[(v * NSL + s) % 6]
                    if eng is nc.scalar:
                        eng.copy(yo[:, sli], ps[:])
                    else:
                        eng.tensor_copy(yo[:, sli], ps[:])

            # ---- rp chunks computed DIRECTLY: the transposed p~ chunk is
            # xt_chunk.T @ wp, one [64,32] matmul per (c, b) — no pp SBUF
            # staging, no PE transposes. The result is duplicated into both
            # partition halves so comp1 matmuls (lhsT base partition 64)
            # have a matching-base rhs.
            rptpA = bpsum.tile([64, 512], f32, tag="bp", name="rptpA")
            rptpB = bpsum.tile([64, 256], f32, tag="bp", name="rptpB")
            for bb in range(B):
                for c in range(2):
                    nc.tensor.matmul(
                        rptpA[:, (c * 8 + bb) * 32:(c * 8 + bb) * 32 + 32],
                        xt[:, bb * 192 + c * 64: bb * 192 + c * 64 + 64],
                        wp_t[:], start=True, stop=True,
                    )
            nc.scalar.copy(rpAB1[64:128, :], rptpA[:])
            nc.vector.tensor_copy(rpAB0[:], rptpA[:])
            for bb in range(B):
                nc.tensor.matmul(
                    rptpB[:, bb * 32:bb * 32 + 32],
                    xt[:, bb * 192 + 128: bb * 192 + 192],
                    wp_t[:], start=True, stop=True,
                )
            nc.vector.tensor_copy(rpC0[:], rptpB[:])
            nc.scalar.copy(rpC1[64:128, :], rptpB[:])
            rp_r = {
                (0, 0): rpAB0[:].rearrange("p (q vr) -> p q vr", q=16, vr=32),
                (0, 1): rpAB1[:].rearrange("p (q vr) -> p q vr", q=16, vr=32),
                (1, 0): rpC0[:].rearrange("p (q vr) -> p q vr", q=8, vr=32),
                (1, 1): rpC1[:].rearrange("p (q vr) -> p q vr", q=8, vr=32),
            }

            def rp(c, comp, v, r2):
                # [64, 8, 1] AP at base partition comp*64: rows i, cols b
                # (stride 32) for fixed (c, 2v+comp)
                rr = rp_r[(1 if c == 2 else 0, comp)]
                q0 = 0 if c == 2 else c * 8
                return rr[comp * 64:comp * 64 + 64,
                          q0:q0 + 8,
                          2 * v + comp:2 * v + comp + 1]

            # ---- s0/s1: comp-split K=64 matmuls. An accumulation group may
            # not mix lhsT base partitions, so comp0 and comp1 accumulate
            # into separate PSUM tiles, summed during the PSUM->SBUF stage
            # (copy from c0, then tensor_tensor add of c1; only one PSUM
            # input is allowed per DVE op).
            s0c = [bpsum.tile([128, 256], f32, tag="bp", name=f"s0c{h}")
                   for h in range(2)]
            s1c = [bpsum.tile([64, 256], f32, tag="bp", name=f"s1c{h}")
                   for h in range(2)]
            # s0 block first -> its PSUM->SBUF chain overlaps the s1 block.
            # comp-MAJOR emission: the comp0 stream needs only the earlier
            # DVE rp copy, so it runs while Act's comp1 rp copy lands.
            for comp in range(2):
                rows = slice(comp * 64, comp * 64 + 64)
                for v in range(NVC):
                    wbv = wbt[v // 8]
                    for r2 in range(2):
                        cs = slice(v * 16 + r2 * 8, v * 16 + r2 * 8 + 8)
                        base = (2 * (v % 8) + r2) * 256
                        nc.tensor.matmul(
                            s0c[comp][:, cs], wbv[rows, base:base + 128],
                            rp(0, comp, v, r2), start=True, stop=False,
                        )
                        nc.tensor.matmul(
                            s0c[comp][:, cs], wbv[rows, base + 128:base + 256],
                            rp(1, comp, v, r2), start=False, stop=True,
                        )

            # batched (v,r2,b) -> (b,v,r2) permute: copy comp0, add comp1
            alu_add = mybir.AluOpType.add
            dst0 = ss0_all[:].rearrange("p (b v r) -> p b v r", b=8, v=16, r=2)
            nc.scalar.copy(
                dst0, s0c[0][:].rearrange("p (v r b) -> p b v r", v=16, r=2, b=8)
            )
            nc.vector.tensor_tensor(
                dst0, dst0,
                s0c[1][:].rearrange("p (v r b) -> p b v r", v=16, r=2, b=8),
                alu_add,
            )

            for comp in range(2):
                rows = slice(comp * 64, comp * 64 + 64)
                for v in range(NVC):
                    wbv = wbt[v // 8]
                    wb2v = wb2t[v // 8]
                    for r2 in range(2):
                        cs = slice(v * 16 + r2 * 8, v * 16 + r2 * 8 + 8)
                        base = (2 * (v % 8) + r2) * 256
                        b2 = (v % 8) * 256 + r2 * 128
                        nc.tensor.matmul(
                            s1c[comp][:, cs], wb2v[rows, b2:b2 + 64],
                            rp(0, comp, v, r2), start=True, stop=False,
                        )
                        nc.tensor.matmul(
                            s1c[comp][:, cs], wb2v[rows, b2 + 64:b2 + 128],
                            rp(1, comp, v, r2), start=False, stop=False,
                        )
                        nc.tensor.matmul(
                            s1c[comp][:, cs], wbv[rows, base:base + 64],
                            rp(2, comp, v, r2), start=False, stop=True,
                        )

            emit_A(0, 0)
            emit_A(1, 0)
            emit_A(2, 0)
            emit_A(3, 0)

            # ---- per-batch transposes into sv layout. The psva half (fed
            # by the ss0 chain, which overlapped the s1 matmul block) is
            # emitted BEFORE the ss1 chain so Act/DVE aren't head-of-line
            # blocked when s1 finishes; the ss1->psvb->inb tail then runs in
            # b-halves to pipeline copy/add with the transposes.
            psva = [bpsum.tile([32, 512], f16, tag="bp", name=f"psva{h}")
                    for h in range(2)]
            psvb = bpsum.tile([32, 512], f16, tag="bp", name="psvb")
            dst = sv_all[:].rearrange("p (b n) -> p b n", b=8, n=192)
            # zero-state col b*192 (only cols never otherwise written)
            nc.vector.memset(dst[:, :, 0:1], 0.0)
            for bb in range(B):
                nc.tensor.transpose(
                    psva[bb // 4][:, (bb % 4) * 128:(bb % 4 + 1) * 128],
                    ss0_all[:, bb * 32:(bb + 1) * 32], id_t[:],
                )
            ina0 = psva[0][:].rearrange("p (b n) -> p b n", b=4, n=128)
            nc.vector.tensor_copy(dst[:, 0:4, 1:129], ina0)
            ina1 = psva[1][:].rearrange("p (b n) -> p b n", b=4, n=128)
            nc.scalar.copy(dst[:, 4:8, 1:129], ina1)

            dst1 = ss1_all[:].rearrange("p (b v r) -> p b v r", b=8, v=16, r=2)
            s1r = [t[:].rearrange("p (v r b) -> p b v r", v=16, r=2, b=8)
                   for t in s1c]
            inb = psvb[:].rearrange("p (b n) -> p b n", b=8, n=64)[:, :, 0:63]
            for h in range(2):
                bs = slice(4 * h, 4 * h + 4)
                nc.scalar.copy(dst1[:, bs], s1r[0][:, bs])
                nc.vector.tensor_tensor(
                    dst1[:, bs], dst1[:, bs], s1r[1][:, bs], alu_add,
                )
                for bb in range(4 * h, 4 * h + 4):
                    nc.tensor.transpose(
                        psvb[:, bb * 64:(bb + 1) * 64],
                        ss1_all[:, bb * 32:(bb + 1) * 32], id_t[0:64, 0:64],
                    )
                nc.vector.tensor_copy(dst[:, bs, 129:192], inb[:, bs])

            # ---- phases A + C per 512-col slice, SLICE-MAJOR: deps are
            # column-range based, and s=0 reads only the earliest-ready sv
            # region (b0-2), so all 16 filters' s=0 slices run while the sv
            # tail writers (ina1/inb b4-7) land. PSUM copies rotate DVE/Act
            # (the only engines that can read PSUM); out DMAs rotate SP/Pool.
            e_out = (nc.sync, nc.gpsimd) * 8
            e_out2 = (nc.gpsimd, nc.sync) * 8
            oeng = (nc.sync, nc.gpsimd, nc.sync)
            # After sv is built the 4 bpsum banks are dead; alternating the
            # stream's PSUM tiles across BOTH pools gives 8 cycling banks so
            # PE never stalls waiting for a copy to free a bank.
            for s in range(NSL):
                for v in range(NVC):
                    if (v, s) not in ps_tiles:
                        emit_A(v, s,
                               pool=bpsum if (v * NSL + s) % 2 else opsum)
                    emit_C(v, s)
                    if v == NVC - 1:
                        # tail vc: per-slice DMA right behind its copy
                        oeng[s].dma_start(
                            out_d[v, :, s * SLW:(s + 1) * SLW],
                            yo_tiles[v][s][:],
                        )
                    elif s == NSL - 1:
                        yo = yo_tiles[v]
                        e_out[v].dma_start(out_d[v, :, 0:768], yo[:, 0:768])
                        e_out2[v].dma_start(out_d[v, :, 768:1536],
                                            yo[:, 768:1536])

    nc.compile()
    return nc


def _get_program():
    if "nc" not in _PROGRAM_CACHE:
        _PROGRAM_CACHE["nc"] = build_nc()
    return _PROGRAM_CACHE["nc"]


# --------------------------------------------------------------------------
# host driver
# --------------------------------------------------------------------------

def make_in_maps(x, a_coeffs, b_coeffs):
    x = np.asarray(x, np.float32)
    a = np.asarray(a_coeffs, np.float64)
    b = np.asarray(b_coeffs, np.float64)
    xf = x[:, 0, :]

    import ml_dtypes

    def to_rhs(x2d):
        xpad = np.zeros((B, TPAD), np.float32)
        xpad[:, :T] = x2d
        xr = np.ascontiguousarray(
            xpad.reshape(B, NBLK, L).transpose(2, 0, 1).reshape(128, NCOL)
        )
        # DoubleRow rhs: [k2, (s, i, n512)] with K = i*64 + k2
        xr8 = np.ascontiguousarray(
            xr.reshape(2, 64, 3, SLW).transpose(1, 2, 0, 3).reshape(64, 2 * NCOL)
        ).astype(ml_dtypes.float8_e4m3)
        return xr.astype(np.float16), xr8

    Xf, X8f = to_rhs(xf)
    Xb, X8b = to_rhs(xf[:, ::-1])
    ident = np.eye(128, dtype=np.float16)

    in_maps = []
    for core in range(8):
        fwd = core < 4
        chans = list(range((core % 4) * NVC, (core % 4) * NVC + NVC))
        tabs = _tables_for_channels(a, b, chans)
        in_maps.append(
            {
                "xrhs": Xf if fwd else Xb,
                "xrhs8": X8f if fwd else X8b,
                "ident": ident,
                **tabs,
            }
        )
    return in_maps


def assemble_output(core_outs):
    y = np.zeros((B, 2 * C, T), np.float32)
    for core in range(8):
        o = np.asarray(core_outs[core]).astype(np.float32) * BETA  # [16, 128, 1536]
        o = o.reshape(NVC, 128, B, NBLK).transpose(2, 0, 3, 1).reshape(B, NVC, TPAD)
        if core < 4:
            y[:, core * NVC:(core + 1) * NVC, :] = o[:, :, :T]
        else:
            y[:, C + (core - 4) * NVC:C + (core - 3) * NVC, :] = o[:, :, :T][:, :, ::-1]
    return y


def kernel(x, a_coeffs, b_coeffs, _trace=False):
    from concourse.bass_utils import run_bass_kernel_spmd

    nc = _get_program()
    in_maps = make_in_maps(x, a_coeffs, b_coeffs)
    res = run_bass_kernel_spmd(
        nc, in_maps, core_ids=list(range(8)), trace=_trace
    )
    y = assemble_output([r["out"] for r in res.results])
    if _trace:
        kernel.last_results = res
    return y
